# revision 1
# baseline (speedup 1.0000x reference)
"""GRU-style cell (nn_Lstmcell) on 8 Trainium2 NeuronCores.

h = (1-z)*h_prev + z*tanh((r*h_prev)@whh + x@whx + bh)
r = sigmoid([x,h_prev]@wr + br),  z = sigmoid([x,h_prev]@wz + bz)

Data-parallel over the batch dim: each of the 8 cores gets B/8 rows; the
small weight matrices are replicated.

Per-core dataflow (feature-major compute):
  - DMA x/h chunks in row-major (batch on partitions, 1KB/partition lines).
  - PE-transpose 128x128 pieces of x and h into feature-major (float32r,
    staged through PSUM, evicted to SBUF).
  - Gates r^T, z^T: float32r matmuls, weights stationary, activations
    streaming with N=512 free dim (f32r streams 1 cy/row at N>=256).
  - sigmoid/tanh + per-partition bias on ScalarE straight out of PSUM.
  - rh = r*h, blend on VectorE, all [128, 1024] feature-major ops.
  - PE-transpose h_out back to batch-major, DMA out.
"""

import numpy as np

import concourse.bass as bass
import concourse.bacc as bacc
import concourse.mybir as mybir
import concourse.tile as tile
from concourse.bass_utils import run_bass_kernel_spmd

NCORES = 8
IN = 256
H = 256
CONCAT = IN + H

F32 = mybir.dt.float32
F32R = mybir.dt.float32r
BF16 = mybir.dt.bfloat16
SIG = mybir.ActivationFunctionType.Sigmoid
TANH = mybir.ActivationFunctionType.Tanh

_BUILD_CACHE = {}
LAST_RESULTS = None


def _r(ap):
    return ap.bitcast(F32R)


def _build(R, reps=1):
    """Build + compile the per-core kernel for R batch rows per core."""
    CHUNK = 1024 if R % 1024 == 0 else 512
    assert R % CHUNK == 0 and CHUNK % 512 == 0
    n_chunks = R // CHUNK
    sub_per_chunk = CHUNK // 128          # 128-row subtiles per chunk
    macros_per_chunk = CHUNK // 512       # 512-row macros per chunk

    nc = bacc.Bacc(
        "TRN2", target_bir_lowering=False, debug=False, num_devices=NCORES
    )

    x_d = nc.dram_tensor("x", [R, IN], F32, kind="ExternalInput").ap()
    h_d = nc.dram_tensor("h_prev", [R, H], F32, kind="ExternalInput").ap()
    wr_d = nc.dram_tensor("wr", [CONCAT, H], F32, kind="ExternalInput").ap()
    wz_d = nc.dram_tensor("wz", [CONCAT, H], F32, kind="ExternalInput").ap()
    whh_d = nc.dram_tensor("whh", [H, H], F32, kind="ExternalInput").ap()
    whx_d = nc.dram_tensor("whx", [IN, H], F32, kind="ExternalInput").ap()
    br_d = nc.dram_tensor("br", [H], F32, kind="ExternalInput").ap()
    bz_d = nc.dram_tensor("bz", [H], F32, kind="ExternalInput").ap()
    bh_d = nc.dram_tensor("bh", [H], F32, kind="ExternalInput").ap()
    id_d = nc.dram_tensor("ident", [128, 128], F32, kind="ExternalInput").ap()
    out_d = nc.dram_tensor("h_out", [R, H], F32, kind="ExternalOutput").ap()

    x_dram = x_d.rearrange("(n p) f -> p n f", p=128)
    h_dram = h_d.rearrange("(n p) f -> p n f", p=128)
    out_dram = out_d.rearrange("(n p) f -> p n f", p=128)

    with tile.TileContext(nc) as tc:
        with (
            tc.tile_pool(name="const", bufs=1) as cpool,
            tc.tile_pool(name="io", bufs=2) as iopool,
            tc.tile_pool(name="work", bufs=2) as wpool,
            tc.tile_pool(name="psg", bufs=1, space="PSUM") as psg,
            tc.tile_pool(name="pst", bufs=1, space="PSUM") as pst,
        ):
            ident = cpool.tile([128, 128], F32R)
            nc.sync.dma_start(ident[:], _r(id_d))
            ident_bf = cpool.tile([128, 128], BF16)
            nc.gpsimd.dma_start(ident_bf[:], id_d)
            wr_sb = cpool.tile([128, 4 * H], BF16)
            nc.gpsimd.dma_start(
                wr_sb[:].rearrange("p (c j) -> p c j", j=H),
                wr_d.rearrange("(c p) j -> p c j", p=128),
            )
            wz_sb = cpool.tile([128, 4 * H], BF16)
            nc.gpsimd.dma_start(
                wz_sb[:].rearrange("p (c j) -> p c j", j=H),
                wz_d.rearrange("(c p) j -> p c j", p=128),
            )
            whh_sb = cpool.tile([128, 2 * H], BF16)
            nc.gpsimd.dma_start(
                whh_sb[:].rearrange("p (c j) -> p c j", j=H),
                whh_d.rearrange("(c p) j -> p c j", p=128),
            )
            whx_sb = cpool.tile([128, 2 * H], BF16)
            nc.gpsimd.dma_start(
                whx_sb[:].rearrange("p (c j) -> p c j", j=H),
                whx_d.rearrange("(c p) j -> p c j", p=128),
            )
            br_sb = cpool.tile([128, 2], F32)
            nc.sync.dma_start(br_sb[:], br_d.rearrange("(c p) -> p c", p=128))
            bz_sb = cpool.tile([128, 2], F32)
            nc.sync.dma_start(bz_sb[:], bz_d.rearrange("(c p) -> p c", p=128))
            bh_sb = cpool.tile([128, 2], F32)
            nc.sync.dma_start(bh_sb[:], bh_d.rearrange("(c p) -> p c", p=128))

            for ci in range(n_chunks * reps):
                ci = ci % n_chunks
                x_ch = iopool.tile([128, sub_per_chunk * IN], BF16, tag="x")
                nc.gpsimd.dma_start(
                    x_ch[:].rearrange("p (n f) -> p n f", f=IN),
                    x_dram[:, ci * sub_per_chunk : (ci + 1) * sub_per_chunk, :],
                )
                h_ch = iopool.tile([128, sub_per_chunk * H], F32R, tag="h")
                nc.sync.dma_start(
                    h_ch[:].rearrange("p (n f) -> p n f", f=H),
                    _r(h_dram[:, ci * sub_per_chunk : (ci + 1) * sub_per_chunk, :]),
                )
                o_ch = iopool.tile([128, sub_per_chunk * H], F32, tag="o")

                for m in range(macros_per_chunk):
                    t0 = m * 4  # first 128-row subtile of this macro

                    # --- transpose x (bf16), h (f32r) into feature-major ---
                    xT = wpool.tile([128, 1024], BF16, tag="xT")
                    stgb = pst.tile([128, 1024], BF16, tag="stgb")
                    for c in range(2):
                        for t in range(4):
                            piece = x_ch[
                                :,
                                (t0 + t) * 256 + c * 128 : (t0 + t) * 256
                                + c * 128
                                + 128,
                            ]
                            nc.tensor.transpose(
                                stgb[:, c * 512 + t * 128 : c * 512 + t * 128 + 128],
                                piece,
                                ident_bf[:],
                            )
                    nc.any.tensor_copy(xT[:], stgb[:])
                    hT = wpool.tile([128, 1024], F32R, tag="hT")
                    for c in range(2):  # feature chunk
                        stg = pst.tile([128, 512], F32R, tag="stg")
                        for t in range(4):  # batch subtile
                            piece = h_ch[
                                :,
                                (t0 + t) * 256 + c * 128 : (t0 + t) * 256
                                + c * 128
                                + 128,
                            ]
                            nc.tensor.transpose(
                                stg[:, t * 128 : (t + 1) * 128],
                                piece,
                                ident[:],
                            )
                        nc.any.tensor_copy(
                            hT[:, c * 512 : (c + 1) * 512], stg[:]
                        )
                    hT_bf = wpool.tile([128, 1024], BF16, tag="hTbf")
                    nc.vector.tensor_copy(hT_bf[:], hT[:].bitcast(F32))

                    def xc_chunk(c):
                        # feature chunk c of [x; h]^T (c in 0..3), bf16
                        sb = xT if c < 2 else hT_bf
                        cc = c % 2
                        return sb[:, cc * 512 : (cc + 1) * 512]

                    # --- gates r^T, z^T: [j, b] = sum_k w[k, j] * xc^T[k, b]
                    ps_r = psg.tile([128, 1024], F32, tag="pr")
                    ps_z = psg.tile([128, 1024], F32, tag="pz")
                    for ps, w_sb in ((ps_r, wr_sb), (ps_z, wz_sb)):
                        for jc in range(2):
                            for c in range(4):
                                nc.tensor.matmul(
                                    ps[:, jc * 512 : (jc + 1) * 512],
                                    w_sb[
                                        :,
                                        c * 256 + jc * 128 : c * 256
                                        + jc * 128
                                        + 128,
                                    ],
                                    xc_chunk(c),
                                    start=(c == 0),
                                    stop=(c == 3),
                                )

                    r_sb = wpool.tile([128, 1024], F32, tag="r")
                    z_sb = wpool.tile([128, 1024], F32, tag="z")
                    for jc in range(2):
                        nc.scalar.activation(
                            r_sb[:, jc * 512 : (jc + 1) * 512],
                            ps_r[:, jc * 512 : (jc + 1) * 512],
                            SIG,
                            bias=br_sb[:, jc : jc + 1],
                        )
                        nc.scalar.activation(
                            z_sb[:, jc * 512 : (jc + 1) * 512],
                            ps_z[:, jc * 512 : (jc + 1) * 512],
                            SIG,
                            bias=bz_sb[:, jc : jc + 1],
                        )

                    # --- rh = r * h (feature-major) ---
                    rh_sb = wpool.tile([128, 1024], BF16, tag="rh")
                    nc.vector.tensor_mul(rh_sb[:], r_sb[:], hT[:].bitcast(F32))

                    # --- g^T = tanh(whh^T-part + whx-part + bh) ---
                    ps_g = psg.tile([128, 1024], F32, tag="pg")
                    for jc in range(2):
                        out_sl = ps_g[:, jc * 512 : (jc + 1) * 512]
                        for k in range(2):
                            nc.tensor.matmul(
                                out_sl,
                                whh_sb[
                                    :,
                                    k * 256 + jc * 128 : k * 256
                                    + jc * 128
                                    + 128,
                                ],
                                rh_sb[:, k * 512 : (k + 1) * 512],
                                start=(k == 0),
                                stop=False,
                            )
                        for k in range(2):
                            nc.tensor.matmul(
                                out_sl,
                                whx_sb[
                                    :,
                                    k * 256 + jc * 128 : k * 256
                                    + jc * 128
                                    + 128,
                                ],
                                xT[:, k * 512 : (k + 1) * 512],
                                start=False,
                                stop=(k == 1),
                            )

                    g_sb = wpool.tile([128, 1024], F32, tag="g")
                    for jc in range(2):
                        nc.scalar.activation(
                            g_sb[:, jc * 512 : (jc + 1) * 512],
                            ps_g[:, jc * 512 : (jc + 1) * 512],
                            TANH,
                            bias=bh_sb[:, jc : jc + 1],
                        )

                    # --- blend: ho = h + z*(g - h) (feature-major) ---
                    t1 = wpool.tile([128, 1024], F32, tag="t1")
                    nc.vector.tensor_sub(t1[:], g_sb[:], hT[:].bitcast(F32))
                    t2 = wpool.tile([128, 1024], F32, tag="t2")
                    nc.vector.tensor_mul(t2[:], z_sb[:], t1[:])
                    ho = wpool.tile([128, 1024], F32R, tag="ho")
                    nc.vector.tensor_add(ho[:], t2[:], hT[:].bitcast(F32))

                    # --- transpose h_out back to batch-major ---
                    for half in range(2):  # batch subtiles (t0+2h, t0+2h+1)
                        stg = pst.tile([128, 512], F32R, tag="stg")
                        for q in range(4):
                            t = half * 2 + q // 2  # subtile within macro
                            jc = q % 2
                            piece = ho[:, jc * 512 + t * 128 : jc * 512 + t * 128 + 128]
                            nc.tensor.transpose(
                                stg[:, q * 128 : (q + 1) * 128],
                                piece,
                                ident[:],
                            )
                        nc.any.tensor_copy(
                            o_ch[
                                :,
                                (t0 + half * 2) * 256 : (t0 + half * 2) * 256 + 512,
                            ],
                            stg[:].bitcast(F32),
                        )

                nc.sync.dma_start(
                    out_dram[:, ci * sub_per_chunk : (ci + 1) * sub_per_chunk, :],
                    o_ch[:].rearrange("p (n f) -> p n f", f=H),
                )

    nc.compile()
    return nc


def kernel(x, h_prev, wr, wz, whh, whx, br, bz, bh):
    global LAST_RESULTS
    x = np.ascontiguousarray(np.asarray(x, dtype=np.float32)).reshape(-1, IN)
    h_prev = np.ascontiguousarray(np.asarray(h_prev, dtype=np.float32)).reshape(
        -1, H
    )
    B = x.shape[0]
    assert B % NCORES == 0
    R = B // NCORES

    if R not in _BUILD_CACHE:
        _BUILD_CACHE[R] = _build(R)
    nc = _BUILD_CACHE[R]

    shared = {
        "wr": np.ascontiguousarray(np.asarray(wr, dtype=np.float32)),
        "wz": np.ascontiguousarray(np.asarray(wz, dtype=np.float32)),
        "whh": np.ascontiguousarray(np.asarray(whh, dtype=np.float32)),
        "whx": np.ascontiguousarray(np.asarray(whx, dtype=np.float32)),
        "br": np.ascontiguousarray(np.asarray(br, dtype=np.float32)),
        "bz": np.ascontiguousarray(np.asarray(bz, dtype=np.float32)),
        "bh": np.ascontiguousarray(np.asarray(bh, dtype=np.float32)),
        "ident": np.eye(128, dtype=np.float32),
    }
    in_maps = []
    for i in range(NCORES):
        m = dict(shared)
        m["x"] = x[i * R : (i + 1) * R]
        m["h_prev"] = h_prev[i * R : (i + 1) * R]
        in_maps.append(m)

    res = run_bass_kernel_spmd(nc, in_maps, list(range(NCORES)))
    LAST_RESULTS = res
    out = np.concatenate([res.results[i]["h_out"] for i in range(NCORES)], axis=0)
    return out.reshape(B, 1, H)



# revision 7
# speedup vs baseline: 2.5081x; 2.5081x over previous
"""GRU-style cell (nn_Lstmcell) on 8 Trainium2 NeuronCores.

h = (1-z)*h_prev + z*tanh((r*h_prev)@whh + x@whx + bh)
r = sigmoid([x,h_prev]@wr + br),  z = sigmoid([x,h_prev]@wz + bz)

Data-parallel over the batch dim: each of the 8 cores gets B/8 rows; the
small weight matrices are replicated. Inputs/weights are fed to the
device as bf16 (host-side cast), halving HBM traffic.

Per-core dataflow — fully feature-major, zero on-chip transposes:
  - x^T, h^T loaded feature-major from DRAM via HWDGE xbar DMA-transpose
    (bf16). The sync ring carries ONLY transposes: mixing plain DMAs
    into the xbar stream costs a multi-us completion handshake per
    class transition, so the single packed weight DMA rides the scalar
    ring and the output is stored once at the end.
  - r^T, z^T, g^T: weights stationary (packed into one SBUF tile),
    activations stream with N=512 into one f32 PSUM bank per matmul.
  - sigmoid/tanh + per-partition bias on ScalarE straight out of PSUM
    (bf16 out); rh and the gated blend on VectorE in bf16 (2x mode).
  - h_out^T accumulates in a resident SBUF buffer; one 2MB store at the
    end. The host transposes back to batch-major (cheap numpy view).
"""

import numpy as np
import ml_dtypes

import concourse.bacc as bacc
import concourse.mybir as mybir
import concourse.tile as tile
from concourse.bass_utils import run_bass_kernel_spmd

NCORES = 8
IN = 256
H = 256
CONCAT = IN + H
CH = 1024  # batch rows per chunk

F32 = mybir.dt.float32
BF16 = mybir.dt.bfloat16
SIG = mybir.ActivationFunctionType.Sigmoid
TANH = mybir.ActivationFunctionType.Tanh

WCAT_COLS = 12 * H + 8  # wr|wz|whh|whx folded + br|bz|bh bias columns

_BUILD_CACHE = {}
LAST_RESULTS = None


def _build(R):
    """Build + compile the per-core kernel for R batch rows per core."""
    assert R % CH == 0
    n_chunks = R // CH

    nc = bacc.Bacc(
        "TRN2", target_bir_lowering=False, debug=False, num_devices=NCORES
    )

    x_d = nc.dram_tensor("x", [R, IN], BF16, kind="ExternalInput").ap()
    h_d = nc.dram_tensor("h_prev", [R, H], BF16, kind="ExternalInput").ap()
    wcat_d = nc.dram_tensor("wcat", [128, WCAT_COLS], BF16, kind="ExternalInput").ap()
    out_d = nc.dram_tensor("h_outT", [2 * H // 2, R], BF16, kind="ExternalOutput").ap()

    with tile.TileContext(nc) as tc:
        with (
            tc.tile_pool(name="const", bufs=1) as cpool,
            tc.tile_pool(name="io", bufs=3) as iopool,
            tc.tile_pool(name="work", bufs=2) as wpool,
            tc.tile_pool(name="pr", bufs=2, space="PSUM") as prpool,
            tc.tile_pool(name="pz", bufs=2, space="PSUM") as pzpool,
            tc.tile_pool(name="pg", bufs=2, space="PSUM") as pgpool,
        ):
            wcat_sb = cpool.tile([128, WCAT_COLS], BF16)
            nc.scalar.dma_start(wcat_sb[:], wcat_d)
            wr_sb = wcat_sb[:, 0 : 4 * H]
            wz_sb = wcat_sb[:, 4 * H : 8 * H]
            br_sb = wcat_sb[:, 12 * H + 0 : 12 * H + 2]
            bz_sb = wcat_sb[:, 12 * H + 2 : 12 * H + 4]
            bh_sb = wcat_sb[:, 12 * H + 4 : 12 * H + 6]

            # whole-run h_out^T accumulator: [p, (jc, b)]
            oT_all = cpool.tile([128, 2 * R], BF16)

            for ci in range(n_chunks):
                b0 = ci * CH
                # --- feature-major loads via xbar DMA-transpose (sync ring
                # only carries these) ---
                xT = iopool.tile([128, 2 * CH], BF16, tag="xT")
                hT = iopool.tile([128, 2 * CH], BF16, tag="hT")
                for kc in range(2):
                    nc.sync.dma_start(
                        xT[:, kc * CH : (kc + 1) * CH],
                        x_d[b0 : b0 + CH, kc * 128 : (kc + 1) * 128],
                        transpose=True,
                    )
                    nc.sync.dma_start(
                        hT[:, kc * CH : (kc + 1) * CH],
                        h_d[b0 : b0 + CH, kc * 128 : (kc + 1) * 128],
                        transpose=True,
                    )

                def xc_sl(kc, lo, n):
                    # feature-major slice of [x; h]^T, chunk kc in 0..3
                    sb = xT if kc < 2 else hT
                    c = kc % 2
                    return sb[:, c * CH + lo : c * CH + lo + n]

                def gate(w_sb, pool, out_sb, func, bias, chunks, korder):
                    # out^T[jc*128+p, b] = func(sum_k w[k,j]*act[k,b] + bias)
                    for jc in range(2):
                        for hf in range(2):
                            ps = pool.tile([128, 512], F32, tag="ps", name="ps")
                            for i, kc in enumerate(korder):
                                nc.tensor.matmul(
                                    ps[:],
                                    w_sb[
                                        :,
                                        kc * H + jc * 128 : kc * H + jc * 128 + 128,
                                    ],
                                    chunks(kc, hf * 512, 512),
                                    start=(i == 0),
                                    stop=(i == len(korder) - 1),
                                )
                            nc.scalar.activation(
                                out_sb[
                                    :, jc * CH + hf * 512 : jc * CH + hf * 512 + 512
                                ],
                                ps[:],
                                func,
                                bias=bias[:, jc : jc + 1],
                            )

                # --- r^T: matmul kc order follows transpose arrival order ---
                r_sb = wpool.tile([128, 2 * CH], BF16, tag="r")
                gate(wr_sb, prpool, r_sb, SIG, br_sb, xc_sl, (0, 2, 1, 3))

                # --- z^T (independent of r; covers the r->rh latency) ---
                z_sb = wpool.tile([128, 2 * CH], BF16, tag="z")
                gate(wz_sb, pzpool, z_sb, SIG, bz_sb, xc_sl, (0, 2, 1, 3))

                # --- rh = r * h^T (feature-major, bf16 2x) ---
                rh = wpool.tile([128, 2 * CH], BF16, tag="rh")
                for jc in range(2):
                    nc.vector.tensor_mul(
                        rh[:, jc * CH : (jc + 1) * CH],
                        r_sb[:, jc * CH : (jc + 1) * CH],
                        hT[:, jc * CH : (jc + 1) * CH],
                    )

                # --- g^T = tanh(rh@whh + x@whx + bh) ---
                def g_sl(kc, lo, n):
                    sb = rh if kc < 2 else xT
                    c = kc % 2
                    return sb[:, c * CH + lo : c * CH + lo + n]

                g_sb = wpool.tile([128, 2 * CH], BF16, tag="g")
                gate(
                    wcat_sb[:, 8 * H : 12 * H],
                    pgpool,
                    g_sb,
                    TANH,
                    bh_sb,
                    g_sl,
                    (0, 2, 1, 3),
                )

                # --- blend feature-major: ho = h + z*(g - h), bf16 2x ---
                d_sb = wpool.tile([128, 2 * CH], BF16, tag="d")
                e_sb = wpool.tile([128, 2 * CH], BF16, tag="e")
                for jc in range(2):
                    sl = slice(jc * CH, (jc + 1) * CH)
                    osl = slice(jc * R + b0, jc * R + b0 + CH)
                    nc.vector.tensor_sub(d_sb[:, sl], g_sb[:, sl], hT[:, sl])
                    nc.vector.tensor_mul(e_sb[:, sl], z_sb[:, sl], d_sb[:, sl])
                    nc.vector.tensor_add(oT_all[:, osl], e_sb[:, sl], hT[:, sl])

            # one store at the very end: no plain DMA ever interleaves with
            # the transpose stream
            nc.gpsimd.dma_start(
                out_d.rearrange("(c p) b -> p c b", p=128),
                oT_all[:].rearrange("p (c b) -> p c b", b=R),
            )

    nc.compile()
    return nc


def _bf16(a):
    return np.ascontiguousarray(np.asarray(a, dtype=np.float32)).astype(
        ml_dtypes.bfloat16
    )


def kernel(x, h_prev, wr, wz, whh, whx, br, bz, bh):
    global LAST_RESULTS
    x = _bf16(x).reshape(-1, IN)
    h_prev = _bf16(h_prev).reshape(-1, H)
    B = x.shape[0]
    assert B % NCORES == 0
    R = B // NCORES

    if R not in _BUILD_CACHE:
        _BUILD_CACHE[R] = _build(R)
    nc = _BUILD_CACHE[R]

    def _fold(w, nchunk):
        w = _bf16(w)
        return w.reshape(nchunk, 128, H).transpose(1, 0, 2).reshape(128, nchunk * H)

    def _bias_fold(b):
        # [H] -> per-partition [128, 2] feature-major (jc chunks)
        return _bf16(b).reshape(2, 128).T

    wcat = np.zeros((128, WCAT_COLS), dtype=ml_dtypes.bfloat16)
    wcat[:, 0 : 4 * H] = _fold(wr, 4)
    wcat[:, 4 * H : 8 * H] = _fold(wz, 4)
    wcat[:, 8 * H : 10 * H] = _fold(whh, 2)
    wcat[:, 10 * H : 12 * H] = _fold(whx, 2)
    wcat[:, 12 * H + 0 : 12 * H + 2] = _bias_fold(br)
    wcat[:, 12 * H + 2 : 12 * H + 4] = _bias_fold(bz)
    wcat[:, 12 * H + 4 : 12 * H + 6] = _bias_fold(bh)
    wcat = np.ascontiguousarray(wcat)

    in_maps = []
    for i in range(NCORES):
        in_maps.append(
            {
                "wcat": wcat,
                "x": x[i * R : (i + 1) * R],
                "h_prev": h_prev[i * R : (i + 1) * R],
            }
        )

    res = run_bass_kernel_spmd(nc, in_maps, list(range(NCORES)))
    LAST_RESULTS = res
    # h_outT is [256, R] feature-major; transpose back on the host
    out = np.concatenate(
        [
            np.asarray(res.results[i]["h_outT"], dtype=np.float32).T
            for i in range(NCORES)
        ],
        axis=0,
    )
    return np.ascontiguousarray(out).reshape(B, 1, H)
